# revision 21
# baseline (speedup 1.0000x reference)
"""DeformBottleneck Trainium2 kernel (Bass/Tile), batch-parallel over 8 cores.

Per core (1 sample, CM=128, HxW=80x80):
  1. y1 = SiLU(BN1(conv1x1(x)))            -- PE + ACT (BN folded on host)
  2. pred = conv3x3(y1)+b (offsets/mask)   -- PE, col-tiled 4 blocks
  3. per-site floor/frac/clip/weights      -- DVE, partition-aligned
  4. wrap-transpose per-site quantities    -- PE transpose -> [site%128, site//128]
  5. Z_n^T = (Wd_n @ y1)^T (9 taps), bf16  -- PE, DMA to DRAM pixel-major
  6. gather pixel-pair rows of Z_n^T       -- SWDGE dma_gather
  7. 4-corner weighted accumulate          -- DVE scalar_tensor_tensor
  8. y2 = SiLU(acc^T); y3 = SiLU(conv1x1); out = x + y3

Uses linearity of bilinear sampling: deform(y1;W) = sum_{n,corner} w*(W_n@y1)[:,idx],
so sampling happens on Z_n = W_n@y1 rows and the only elementwise work is the
4-corner weighted sum, done with per-partition-scalar fused MACs in a
sites-on-partitions layout.
"""

import sys

if "/opt/trn_rl_repo" not in sys.path:
    sys.path.insert(0, "/opt/trn_rl_repo")

import contextlib

import numpy as np

import concourse.bass as bass
import concourse.mybir as mybir
import concourse.tile as tile
from concourse import bacc, bass_utils
from concourse.masks import make_identity

F32 = mybir.dt.float32
BF16 = mybir.dt.bfloat16
I16 = mybir.dt.int16
I32 = mybir.dt.int32
AF = mybir.ActivationFunctionType
OP = mybir.AluOpType

EPS = 1e-5
CM = 128
C1 = 256
NTAP = 9

# wrapped-quantity column order in QT (post-transpose):
# per tap n: wy0,wy1,wx0,wx1,mask,fc0y,fc1y,fc0x at cols q*9+n
QWY0, QWY1, QWX0, QWX1, QMSK, QF0Y, QF1Y, QF0X = range(8)
NQ = 8 * NTAP  # 72


def _tap_dydx(n):
    return n // 3 - 1, n % 3 - 1


def build_nc(H, W, rt, dbg=False):
    """rt = image rows per conv tile. Requires (H/rt)%4==0, rt*W<=512,
    H*W%128==0, H*W%16==0."""
    HW = H * W
    T = H // rt
    assert T % 4 == 0 and rt * W <= 512 and HW % 128 == 0
    R = T // 4
    TW = rt * W
    NC = HW // 128
    IW = HW // 16

    nc = bacc.Bacc("TRN2", target_bir_lowering=False, debug=False)

    x_d = nc.dram_tensor("x", [C1, HW], F32, kind="ExternalInput")
    w1T_d = nc.dram_tensor("w1T", [2, 128, 128], F32, kind="ExternalInput")
    b1_d = nc.dram_tensor("b1", [128, 1], F32, kind="ExternalInput")
    offwT_d = nc.dram_tensor("offwT", [NTAP, 128, 27], F32, kind="ExternalInput")
    offbm_d = nc.dram_tensor("offbm", [128, 1], F32, kind="ExternalInput")
    basep_d = nc.dram_tensor("basep", [128, R, TW], F32, kind="ExternalInput")
    cliphi_d = nc.dram_tensor("cliphi", [128, 1], F32, kind="ExternalInput")
    wdT_d = nc.dram_tensor("wdT", [NTAP, 128, 128], F32, kind="ExternalInput")
    b2bc_d = nc.dram_tensor("b2bc", [128, 128], F32, kind="ExternalInput")
    w3T_d = nc.dram_tensor("w3T", [2, 128, 128], F32, kind="ExternalInput")
    b3_d = nc.dram_tensor("b3", [128, 2], F32, kind="ExternalInput")
    zt_d = nc.dram_tensor("zt", [NTAP, HW + 2, 128], BF16, kind="Internal")
    out_d = nc.dram_tensor("out", [C1, HW], F32, kind="ExternalOutput")
    zt_t = zt_d.ap().tensor
    if dbg:
        dbg_y1 = nc.dram_tensor("dbg_y1", [128, HW], F32, kind="ExternalOutput")
        dbg_pred = nc.dram_tensor("dbg_pred", [128, R * TW], F32, kind="ExternalOutput")
        dbg_qs = nc.dram_tensor("dbg_qs", [NQ, HW], F32, kind="ExternalOutput")
        dbg_wq = nc.dram_tensor("dbg_wq", [128, NC * 36], F32, kind="ExternalOutput")
        dbg_idx = nc.dram_tensor("dbg_idx", [128, 2 * NTAP * IW], I16, kind="ExternalOutput")
        dbg_acc = nc.dram_tensor("dbg_acc", [128, NC * 128], F32, kind="ExternalOutput")
        dbg_y2 = nc.dram_tensor("dbg_y2", [128, HW], F32, kind="ExternalOutput")

    WP = W + 4  # padded row width for shifted reads

    with tile.TileContext(nc) as tc, contextlib.ExitStack() as est:
        singles = est.enter_context(tc.tile_pool(name="singles", bufs=1))
        pl_w_cm = tc.tile_pool(name="pl_w", bufs=1)
        pl_w = pl_w_cm.__enter__()
        pl_qs_cm = tc.tile_pool(name="pl_qs", bufs=1)
        pl_qs = pl_qs_cm.__enter__()
        pl_y1_cm = tc.tile_pool(name="pl_y1", bufs=1)
        pl_y1 = pl_y1_cm.__enter__()
        pl_mid_cm = tc.tile_pool(name="pl_mid", bufs=1)
        pl_mid = pl_mid_cm.__enter__()
        WQ = pl_w.tile([128, NC, 36], F32)
        idx16 = pl_w.tile([128, 2 * NTAP, IW], I16)
        QS = pl_qs.tile([128, HW], F32)

        ident = singles.tile([128, 128], F32)
        make_identity(nc, ident)
        w1T = singles.tile([128, 2, 128], F32)
        nc.sync.dma_start(out=w1T, in_=w1T_d.ap().rearrange("k p m -> p k m"))
        b1 = singles.tile([128, 1], F32)
        nc.sync.dma_start(out=b1, in_=b1_d.ap())
        offwT = singles.tile([128, NTAP, 27], F32)
        nc.sync.dma_start(out=offwT, in_=offwT_d.ap().rearrange("n p m -> p n m"))
        offbm = singles.tile([128, 1], F32)
        nc.sync.dma_start(out=offbm, in_=offbm_d.ap())
        basep = singles.tile([128, R, TW], F32)
        nc.sync.dma_start(out=basep, in_=basep_d.ap())
        cliphi = singles.tile([128, 1], F32)
        nc.sync.dma_start(out=cliphi, in_=cliphi_d.ap())
        wdT = singles.tile([128, NTAP, 128], F32)
        nc.sync.dma_start(out=wdT, in_=wdT_d.ap().rearrange("n p m -> p n m"))
        b2bc = singles.tile([128, 128], F32)
        nc.sync.dma_start(out=b2bc, in_=b2bc_d.ap())
        w3T = singles.tile([128, 2, 128], F32)
        nc.sync.dma_start(out=w3T, in_=w3T_d.ap().rearrange("k p m -> p k m"))
        b3 = singles.tile([128, 2], F32)
        nc.sync.dma_start(out=b3, in_=b3_d.ap())

        # ---------------- phase 1: conv1x1 + BN1 + SiLU -> y1 --------------
        y1 = pl_y1.tile([128, HW], F32)
        y1p = pl_mid.tile([128, (H + 2) * WP], F32)
        y1p_v = y1p.rearrange("p (h w) -> p h w", w=WP)

        with contextlib.ExitStack() as ph:
            xin = ph.enter_context(tc.tile_pool(name="xin", bufs=1))
            mm1ps = ph.enter_context(tc.tile_pool(name="mm1ps", bufs=4, space="PSUM"))
            x0 = xin.tile([128, HW], F32)
            x1 = xin.tile([128, HW], F32)
            nc.sync.dma_start(out=x0, in_=x_d.ap()[0:128, :])
            nc.sync.dma_start(out=x1, in_=x_d.ap()[128:256, :])
            nc.vector.memset(y1p, 0.0)

            sil = ph.enter_context(tc.tile_pool(name="sil", bufs=4))
            for t in range(T):
                ps = mm1ps.tile([128, TW], F32)
                sl = bass.ts(t, TW)
                nc.tensor.matmul(ps, w1T[:, 0, :], x0[:, sl], start=True, stop=False)
                nc.tensor.matmul(ps, w1T[:, 1, :], x1[:, sl], start=False, stop=True)
                sg = sil.tile([128, TW], F32, tag="sg")
                zz = sil.tile([128, TW], F32, tag="zz")
                nc.scalar.activation(sg, ps, AF.Sigmoid, bias=b1[:, 0:1])
                nc.scalar.activation(zz, ps, AF.Identity, bias=b1[:, 0:1])
                nc.vector.tensor_tensor(y1[:, sl], zz, sg, OP.mult)
                nc.gpsimd.tensor_copy(
                    y1p_v[:, 1 + t * rt : 1 + (t + 1) * rt, 1 : 1 + W],
                    y1[:, sl].rearrange("p (r w) -> p r w", w=W),
                )

        # ------- phase 2: offset conv (col-tiled) + per-site quantities ----
        # pred: partition 32j+q, free (r,u) for conv tile t=4r+j;
        # q 0:9 = py (incl. base+bias), 9:18 = px, 18:27 = sigmoid(mask)
        pred = pl_mid.tile([128, R, TW], F32)
        sgm = pl_mid.tile([128, R, TW], F32)
        nc.vector.memset(pred, 0.0)
        nc.vector.memset(sgm, 0.0)

        with contextlib.ExitStack() as ph:
            offps = ph.enter_context(tc.tile_pool(name="offps", bufs=2, space="PSUM"))
            for r in range(R):
                ps = offps.tile([128, TW], F32)
                for n in range(NTAP):
                    dy, dx = _tap_dydx(n)
                    for j in range(4):
                        t = 4 * r + j
                        rhs = y1p_v[
                            :,
                            1 + t * rt + dy : 1 + (t + 1) * rt + dy,
                            1 + dx : 1 + W + dx,
                        ]
                        nc.tensor.matmul(
                            ps[32 * j : 32 * j + 27, :],
                            offwT[:, n, :],
                            rhs,
                            start=(n == 0),
                            stop=(n == NTAP - 1),
                            tile_position=(0, 32 * j),
                            skip_group_check=True,
                        )
                for j in range(4):
                    # offset rows get +base; mask rows get +0 (raw logits)
                    nc.vector.tensor_tensor(
                        pred[32 * j : 32 * j + 27, r, :],
                        ps[32 * j : 32 * j + 27, :],
                        basep[32 * j : 32 * j + 27, r, :],
                        OP.add,
                    )
                    # sigmoid of the whole block; only mask rows are used
                    nc.scalar.activation(
                        sgm[32 * j : 32 * j + 27, r, :],
                        ps[32 * j : 32 * j + 27, :],
                        AF.Sigmoid,
                        bias=offbm[32 * j : 32 * j + 27, 0:1],
                    )

            # ---- per-site math, all ops partition-aligned on [128, R*TW] --
            sp = ph.enter_context(tc.tile_pool(name="persite", bufs=1))
            pv = pred.rearrange("p r u -> p (r u)")
            Ff = sp.tile([128, R * TW], F32, tag="sA")
            ii = sp.tile([128, R * TW], I32, tag="sI")
            tmp = sp.tile([128, R * TW], F32, tag="sB")
            v0 = sp.tile([128, R * TW], F32, tag="sC")
            v1 = sp.tile([128, R * TW], F32, tag="sD")
            w0 = sp.tile([128, R * TW], F32, tag="sE")
            w1 = sp.tile([128, R * TW], F32, tag="sF")
            fc0 = sp.tile([128, R * TW], F32, tag="sG")
            fc1 = sp.tile([128, R * TW], F32, tag="sH")

            # floor via +16384/trunc, then correct both directions
            nc.vector.tensor_scalar(Ff, pv, 16384.0, None, OP.add)
            nc.vector.tensor_copy(ii, Ff)
            nc.vector.tensor_copy(Ff, ii)
            nc.vector.tensor_scalar(Ff, Ff, 16384.0, None, OP.subtract)
            nc.vector.tensor_tensor(tmp, Ff, pv, OP.is_gt)
            nc.vector.tensor_tensor(Ff, Ff, tmp, OP.subtract)
            nc.vector.tensor_tensor(tmp, pv, Ff, OP.subtract)
            nc.vector.tensor_scalar(tmp, tmp, 1.0, None, OP.is_ge)
            nc.vector.tensor_tensor(Ff, Ff, tmp, OP.add)
            frac = tmp
            nc.vector.tensor_tensor(frac, pv, Ff, OP.subtract)

            nc.vector.tensor_scalar(v0, Ff, 0.0, None, OP.is_ge)
            nc.vector.tensor_scalar(w0, Ff, float(H - 1), None, OP.is_le)
            nc.vector.tensor_tensor(v0, v0, w0, OP.mult)
            nc.vector.tensor_scalar(v1, Ff, -1.0, None, OP.is_ge)
            nc.vector.tensor_scalar(w1, Ff, float(H - 2), None, OP.is_le)
            nc.vector.tensor_tensor(v1, v1, w1, OP.mult)
            nc.vector.tensor_scalar(w0, frac, -1.0, 1.0, OP.mult, OP.add)
            nc.vector.tensor_tensor(w0, w0, v0, OP.mult)
            nc.vector.tensor_tensor(w1, frac, v1, OP.mult)
            nc.vector.tensor_scalar(fc0, Ff, 0.0, None, OP.max)
            nc.vector.tensor_scalar(fc0, fc0, cliphi[:, 0:1], None, OP.min)
            nc.vector.tensor_scalar(fc1, Ff, -1.0, 1.0, OP.max, OP.add)
            nc.vector.tensor_scalar(fc1, fc1, float(H - 1), None, OP.min)
            # pair-gather edge swap: when x0==-1 the left corner value sits in
            # the pair's lo slot; when x0==W-1 the right corner sits in hi.
            wlo = sp.tile([128, R * TW], F32, tag="sB")
            whi = sp.tile([128, R * TW], F32, tag="sI")
            iL, iR = v0, v1  # reuse (consumed into w0/w1 above)
            nc.vector.tensor_scalar(iL, Ff, -1.0, None, OP.is_equal)
            nc.vector.tensor_scalar(iR, Ff, float(W - 1), None, OP.is_equal)
            nc.vector.tensor_scalar(Ff, iR, -1.0, 1.0, OP.mult, OP.add)
            nc.vector.tensor_tensor(wlo, w0, Ff, OP.mult)
            nc.vector.tensor_tensor(Ff, w1, iL, OP.mult)
            nc.vector.tensor_tensor(wlo, wlo, Ff, OP.add)
            nc.vector.tensor_scalar(Ff, iL, -1.0, 1.0, OP.mult, OP.add)
            nc.vector.tensor_tensor(whi, w1, Ff, OP.mult)
            nc.vector.tensor_tensor(Ff, w0, iR, OP.mult)
            nc.vector.tensor_tensor(whi, whi, Ff, OP.add)

            # ---- assemble QS [72, sites] via SBUF->SBUF DMAs --------------
            # QS row q*9+n <- quantity q of tap n; site-major free dim
            sgv = sgm.rearrange("p r u -> p (r u)")
            srcs = {QWY0: w0, QWY1: w1, QWX0: wlo, QWX1: whi, QMSK: sgv,
                    QF0Y: fc0, QF1Y: fc1, QF0X: fc0}
            offs = {QWY0: 0, QWY1: 0, QWX0: 9, QWX1: 9, QMSK: 18,
                    QF0Y: 0, QF1Y: 0, QF0X: 9}
            for q in range(8):
                src_t = srcs[q]
                for j in range(4):
                    src = src_t[32 * j + offs[q] : 32 * j + offs[q] + 9, :]
                    src = src.rearrange("p (r u) -> p r u", u=TW)
                    dst = QS[9 * q : 9 * q + 9, :].rearrange(
                        "p (r j u) -> p r j u", j=4, u=TW
                    )[:, :, j, :]
                    nc.sync.dma_start(out=dst, in_=src)

        pl_mid_cm.__exit__(None, None, None)

        # ------- phase 3: wrap-transpose -> QT; weights WQ; idx lists ------

        with contextlib.ExitStack() as ph:
            tps = ph.enter_context(tc.tile_pool(name="tps", bufs=4, space="PSUM"))
            qtp = ph.enter_context(tc.tile_pool(name="qtp", bufs=1))
            QT = qtp.tile([128, NC, NQ + 2], F32)
            idxq = qtp.tile([128, NC, 18], I16)
            for c in range(NC):
                ps = tps.tile([128, NQ], F32)
                nc.tensor.transpose(ps, QS[0:NQ, bass.ts(c, 128)], ident[0:NQ, 0:NQ])
                nc.vector.tensor_copy(QT[:, c, 0:NQ], ps)

            def col(q, n):
                return QT[:, :, q * NTAP + n]

            sy0, sy1 = QT[:, :, NQ], QT[:, :, NQ + 1]
            for n in range(NTAP):
                nc.vector.tensor_tensor(sy0, col(QWY0, n), col(QMSK, n), OP.mult)
                nc.vector.tensor_tensor(sy1, col(QWY1, n), col(QMSK, n), OP.mult)
                nc.vector.tensor_tensor(WQ[:, :, 0 * NTAP + n], sy0, col(QWX0, n), OP.mult)
                nc.vector.tensor_tensor(WQ[:, :, 1 * NTAP + n], sy0, col(QWX1, n), OP.mult)
                nc.vector.tensor_tensor(WQ[:, :, 2 * NTAP + n], sy1, col(QWX0, n), OP.mult)
                nc.vector.tensor_tensor(WQ[:, :, 3 * NTAP + n], sy1, col(QWX1, n), OP.mult)
                nc.vector.scalar_tensor_tensor(
                    idxq[:, :, n], col(QF0Y, n), float(W), col(QF0X, n),
                    OP.mult, OP.add)
                nc.vector.scalar_tensor_tensor(
                    idxq[:, :, NTAP + n], col(QF1Y, n), float(W), col(QF0X, n),
                    OP.mult, OP.add)

            # fold idxq [128, NC] -> wrapped [16, 8*NC] lists; replicate x8
            for k in range(2 * NTAP):
                for g in range(8):
                    src = idxq[16 * g : 16 * g + 16, :, k]
                    dst = idx16[0:16, k, :].rearrange("p (c g) -> p c g", g=8)[
                        :, :, g
                    ]
                    nc.sync.dma_start(out=dst, in_=src)
            for g in range(1, 8):
                nc.sync.dma_start(
                    out=idx16[16 * g : 16 * g + 16, :, :], in_=idx16[0:16, :, :]
                )

        # zero the 2 pad rows after each zt plane (never gathered, but the
        # interp validates the whole gather AP view)
        ztz = singles.tile([128, 2 * NTAP], BF16)
        nc.vector.memset(ztz, 0.0)
        for n in range(NTAP):
            zpad_dst = bass.AP(
                tensor=zt_t, offset=n * (HW + 2) * 128 + HW * 128,
                ap=[[1, 128], [128, 2]],
            )
            nc.sync.dma_start(out=zpad_dst, in_=ztz[:, 2 * n : 2 * n + 2])

        # ------- phase 4: Z_n^T matmuls -> DRAM bf16 -----------------------
        with contextlib.ExitStack() as ph:
            zps = ph.enter_context(tc.tile_pool(name="zps", bufs=4, space="PSUM"))
            zsb = ph.enter_context(tc.tile_pool(name="zsb", bufs=3))
            for c in range(NC):
                lhs = y1[:, bass.ts(c, 128)]
                zt_sb = zsb.tile([128, NTAP, 128], BF16, tag="zt")
                for q4 in range(3):
                    ps = zps.tile([128, 512], F32, tag="zp")
                    for nm in range(4):
                        n = 4 * q4 + nm
                        if n >= NTAP:
                            break
                        nc.tensor.matmul(
                            ps[:, bass.ts(nm, 128)], lhs, wdT[:, n, :],
                            start=True, stop=True,
                        )
                        nc.scalar.activation(
                            zt_sb[:, n, :], ps[:, bass.ts(nm, 128)], AF.Copy
                        )
                dst = bass.AP(
                    tensor=zt_t,
                    offset=c * 128 * 128,
                    ap=[[128, 128], [(HW + 2) * 128, NTAP], [1, 128]],
                )
                nc.sync.dma_start(out=dst, in_=zt_sb)

        # ------- phase 5: gather + weighted-corner accumulate --------------
        pl_y1_cm.__exit__(None, None, None)
        pl_acc_cm = tc.tile_pool(name="pl_acc", bufs=1)
        pl_acc = pl_acc_cm.__enter__()
        acc = pl_acc.tile([128, NC, 128], F32)
        with contextlib.ExitStack() as ph:
            gp = ph.enter_context(tc.tile_pool(name="gpool", bufs=4))
            for n in range(NTAP):
                g0 = gp.tile([128, NC, 256], BF16, tag="g")
                g1 = gp.tile([128, NC, 256], BF16, tag="g")
                g = [g0, g1]
                src = bass.AP(
                    tensor=zt_t, offset=n * (HW + 2) * 128,
                    ap=[[128, HW], [1, 256]],
                )
                # split into <=1024-index chunks: larger single gathers
                # overflow the SWDGE descriptor ring and wedge the device
                CG = 8  # NC columns per gather chunk (8*128 = 1024 idxs)
                for rr in range(2):
                    for c0 in range(0, NC, CG):
                        cw = min(CG, NC - c0)
                        nc.gpsimd.dma_gather(
                            g[rr][:, c0 : c0 + cw, :],
                            src,
                            idx16[:, rr * NTAP + n, 8 * c0 : 8 * (c0 + cw)],
                            cw * 128, cw * 128, 256, elem_step=128,
                        )
                for c in range(NC):
                    for k in range(4):  # (rr, hilo): w order wy0wx0,wy0wx1,...
                        rr, hl = k // 2, k % 2
                        step = n * 4 + k
                        in1 = b2bc if step == 0 else acc[:, c, :]
                        nc.vector.scalar_tensor_tensor(
                            acc[:, c, :],
                            g[rr][:, c, 128 * hl : 128 * hl + 128],
                            WQ[:, c, k * NTAP + n : k * NTAP + n + 1],
                            in1,
                            OP.mult,
                            OP.add,
                        )

        final_acc = acc

        # ------- phase 6: y2 = SiLU(acc^T); conv1x1 #3 + residual ----------
        y2 = QS  # reuse the QS buffer (dead after phase 3)
        with contextlib.ExitStack() as ph:
            t2ps = ph.enter_context(tc.tile_pool(name="t2ps", bufs=4, space="PSUM"))
            sil2 = ph.enter_context(tc.tile_pool(name="sil2", bufs=4))
            for c in range(NC):
                ps = t2ps.tile([128, 128], F32)
                nc.tensor.transpose(ps, final_acc[:, c, :], ident)
                sg = sil2.tile([128, 128], F32, tag="sg2")
                nc.scalar.activation(sg, ps, AF.Sigmoid)
                nc.vector.tensor_tensor(y2[:, bass.ts(c, 128)], ps, sg, OP.mult)

        if dbg:
            nc.sync.dma_start(out=dbg_y1.ap(), in_=y1)
            nc.sync.dma_start(out=dbg_pred.ap(), in_=pred.rearrange("p r u -> p (r u)"))
            nc.sync.dma_start(out=dbg_qs.ap(), in_=QS[0:NQ, :])
            nc.sync.dma_start(out=dbg_wq.ap(), in_=WQ.rearrange("p c q -> p (c q)"))
            nc.sync.dma_start(out=dbg_idx.ap(), in_=idx16.rearrange("p k w -> p (k w)"))
            nc.sync.dma_start(out=dbg_acc.ap(), in_=final_acc.rearrange("p c o -> p (c o)"))
            nc.sync.dma_start(out=dbg_y2.ap(), in_=y2)

        pl_acc_cm.__exit__(None, None, None)

        with contextlib.ExitStack() as ph:
            m3ps = ph.enter_context(tc.tile_pool(name="m3ps", bufs=4, space="PSUM"))
            xr = ph.enter_context(tc.tile_pool(name="xr", bufs=4))
            for t in range(T):
                sl = bass.ts(t, TW)
                for h in range(2):
                    ps = m3ps.tile([128, TW], F32)
                    nc.tensor.matmul(ps, w3T[:, h, :], y2[:, sl], start=True, stop=True)
                    xt = xr.tile([128, TW], F32, tag="xt")
                    nc.sync.dma_start(
                        out=xt, in_=x_d.ap()[128 * h : 128 * (h + 1), sl]
                    )
                    ot = xr.tile([128, TW], F32, tag="ot")
                    zz = xr.tile([128, TW], F32, tag="zz3")
                    nc.scalar.activation(ot, ps, AF.Sigmoid, bias=b3[:, h : h + 1])
                    nc.scalar.activation(zz, ps, AF.Identity, bias=b3[:, h : h + 1])
                    nc.vector.tensor_tensor(ot, ot, zz, OP.mult)
                    nc.vector.tensor_tensor(ot, ot, xt, OP.add)
                    nc.sync.dma_start(
                        out=out_d.ap()[128 * h : 128 * (h + 1), sl], in_=ot
                    )

        pl_y1_cm2 = None  # (pl_qs, pl_w close with the ExitStack via TileContext exit)
        pl_qs_cm.__exit__(None, None, None)
        pl_w_cm.__exit__(None, None, None)

    nc.compile()
    return nc


def _fold_host(inputs, H, W, rt):
    """Host-side BN folding + packed constant prep (numpy float32)."""
    f = lambda k: np.asarray(inputs[k], np.float32)
    T = H // rt
    R = T // 4
    TW = rt * W

    inv1 = f("bn1_g") / np.sqrt(f("bn1_v") + EPS)
    W1 = inv1[:, None] * f("cv1_w")
    b1 = (f("bn1_b") - f("bn1_m") * inv1)[:, None].copy()
    w1T = np.ascontiguousarray(W1.T).reshape(2, 128, 128)

    off_w = f("off_w")
    off_b = f("off_b")
    perm = np.concatenate(
        [np.arange(0, 18, 2), np.arange(1, 18, 2), np.arange(18, 27)]
    )
    offwT = np.stack(
        [np.ascontiguousarray(off_w[perm, :, n // 3, n % 3].T) for n in range(NTAP)]
    )
    offbm = np.zeros((128, 1), np.float32)
    for j in range(4):
        offbm[32 * j + 18 : 32 * j + 27, 0] = off_b[18:27]

    basep = np.zeros((128, R, TW), np.float32)
    hh, ww = np.meshgrid(
        np.arange(H, dtype=np.float32), np.arange(W, dtype=np.float32), indexing="ij"
    )
    for t in range(T):
        j, r = t % 4, t // 4
        rows = slice(t * rt, (t + 1) * rt)
        hl = hh[rows].reshape(-1)
        wl = ww[rows].reshape(-1)
        for n in range(NTAP):
            dy, dx = _tap_dydx(n)
            basep[32 * j + n, r, :] = hl + dy + off_b[2 * n]
            basep[32 * j + 9 + n, r, :] = wl + dx + off_b[2 * n + 1]

    cliphi = np.zeros((128, 1), np.float32)
    for j in range(4):
        cliphi[32 * j : 32 * j + 9, 0] = H - 1
        cliphi[32 * j + 9 : 32 * j + 18, 0] = W - 2

    inv2 = f("bn2_g") / np.sqrt(f("bn2_v") + EPS)
    Wd = inv2[:, None, None] * f("dcn_w").reshape(CM, CM, NTAP)
    wdT = np.stack([np.ascontiguousarray(Wd[:, :, n].T) for n in range(NTAP)])
    b2 = f("dcn_b") * inv2 + f("bn2_b") - f("bn2_m") * inv2
    b2bc = np.broadcast_to(b2[None, :], (128, 128)).copy()

    inv3 = f("bn3_g") / np.sqrt(f("bn3_v") + EPS)
    W3 = inv3[:, None] * f("cv3_w")
    w3T = np.ascontiguousarray(W3.T).reshape(128, 2, 128).transpose(1, 0, 2).copy()
    b3 = np.ascontiguousarray((f("bn3_b") - f("bn3_m") * inv3).reshape(2, 128).T)

    return dict(
        w1T=w1T, b1=b1, offwT=offwT, offbm=offbm, basep=basep,
        cliphi=cliphi, wdT=wdT, b2bc=b2bc, w3T=w3T, b3=b3,
    )


_NC_CACHE = {}


def _get_nc(H, W, rt):
    key = (H, W, rt)
    if key not in _NC_CACHE:
        _NC_CACHE[key] = build_nc(H, W, rt)
    return _NC_CACHE[key]


def kernel(**inputs):
    H = W = 80
    rt = 5
    B = inputs["x"].shape[0]
    nc = _get_nc(H, W, rt)
    const = _fold_host(inputs, H, W, rt)
    x = np.asarray(inputs["x"], np.float32).reshape(B, C1, H * W)
    in_maps = [dict(const, x=np.ascontiguousarray(x[i])) for i in range(B)]
    res = bass_utils.run_bass_kernel_spmd(nc, in_maps, list(range(B)))
    out = np.stack([res.results[i]["out"] for i in range(B)])
    return out.reshape(B, C1, H, W).astype(np.float32)
